# revision 6
# baseline (speedup 1.0000x reference)
"""Ball-query (CustomNeighborSearch) Trainium2 kernel.

Problem: N=16384 data points, Q=16384 queries in [0,1]^3, radius 0.08.
Outputs: mask [Q,N] bool, row_splits [Q+1] int32, weights [Q,N] f32.

Strategy (query-parallel over 8 NeuronCores, 2048 queries/core):
  sqdist = q2 + d2 - 2 q.d computed as ONE K=5 fp32 matmul on the PE
  (augmented operands), chunked [128 queries x 2048 data] into PSUM.
  ACT: x = Relu(psum) -> SBUF (into the weights row tile).
  DVE: mask_u8 = (x is_le r2) is_ge x   [+ accum_out -> per-chunk counts]
  DVE: w      = (x is_le r2) mult  x    [in-place over x]
  Outputs DMA'd per row-tile (weights in 4MB halves, mask in 2MB rows).
  row_splits assembled on host from exact per-chunk counts (f32 ints).
"""
import numpy as np
from contextlib import ExitStack

import jax
import jax.numpy as jnp
from jax.experimental.shard_map import shard_map
from jax.sharding import Mesh, PartitionSpec

import concourse.bass as bass
import concourse.tile as tile
import bass_rust
from concourse import mybir
from concourse.bass2jax import (
    _bass_exec_p,
    install_neuronx_cc_hook,
    partition_id_tensor,
)

N = 16384          # data points
Q = 16384          # queries
NCORES = 8
QC = Q // NCORES   # queries per core (2048)
QTILE = 128        # queries per tile (partition dim)
GRP = 2048         # free-dim columns processed per op group (4 PSUM banks)
BANK = 512         # fp32 columns per PSUM bank / matmul
NQT = QC // QTILE  # 16 q-tiles per core
NG = N // GRP      # 8 groups per q-tile row

_mw_counter = [0]


def _split_multiwaits(nc):
    """This walrus build supports only one semaphore wait on DMA/Matmult/
    Drain instructions; move extra waits onto same-engine NOPs placed just
    before the instruction (same engine program order, so semantics hold)."""
    for bb in nc.main_func.blocks:
        out = []
        for ins in bb.instructions:
            si = ins.sync_info
            waits = list(si.on_wait) if si and si.on_wait else []
            if len(waits) > 1:
                for w in waits[:-1]:
                    _mw_counter[0] += 1
                    nop = bass_rust.InstNoOp(
                        name=f"I-mwsplit-{_mw_counter[0]}", engine=ins.engine,
                        ins=[], outs=[])
                    nop.sync_info = mybir.SyncInfo(on_wait=[w], on_update=[])
                    out.append(nop)
                ins.sync_info = mybir.SyncInfo(
                    on_wait=[waits[-1]], on_update=list(si.on_update or []))
            out.append(ins)
        bb.instructions = out


def _build(r2: float):
    nc = bass.Bass()
    qd_d = nc.dram_tensor("qd_aug", [5, QC + N], mybir.dt.float32,
                          kind="ExternalInput")
    mask_d = nc.dram_tensor("mask", [QC, N], mybir.dt.uint8,
                            kind="ExternalOutput")
    w_d = nc.dram_tensor("weights", [QC, N], mybir.dt.float32,
                         kind="ExternalOutput")
    cnt_d = nc.dram_tensor("counts", [QTILE, NQT * NG], mybir.dt.float32,
                           kind="ExternalOutput")

    with ExitStack() as ctx:
        tc = ctx.enter_context(tile.TileContext(nc))
        const_pool = ctx.enter_context(tc.tile_pool(name="const", bufs=1))
        psum_pool = ctx.enter_context(tc.tile_pool(name="psum", bufs=2,
                                                   space="PSUM"))
        w_pool = ctx.enter_context(tc.tile_pool(name="wrow", bufs=2))
        m_pool = ctx.enter_context(tc.tile_pool(name="mrow", bufs=2))

        qd = const_pool.tile([5, QC + N], mybir.dt.float32)
        nc.sync.dma_start(qd[:], qd_d[:])
        d_aug = qd[:, QC:]
        counts = const_pool.tile([QTILE, NQT * NG], mybir.dt.float32)

        for t in range(NQT):
            lhsT = qd[:, t * QTILE:(t + 1) * QTILE]
            m_row = m_pool.tile([QTILE, N], mybir.dt.uint8, tag="mrow")
            for h in range(2):  # two weight half-rows per q-tile
                w_half = w_pool.tile([QTILE, N // 2], mybir.dt.float32,
                                     tag="wrow")
                for gi in range(NG // 2):
                    g = h * (NG // 2) + gi
                    ps = psum_pool.tile([QTILE, GRP], mybir.dt.float32,
                                        tag="ps")
                    for j in range(GRP // BANK):
                        nc.tensor.matmul(
                            ps[:, j * BANK:(j + 1) * BANK], lhsT,
                            d_aug[:, g * GRP + j * BANK:
                                  g * GRP + (j + 1) * BANK],
                            start=True, stop=True)
                    x = w_half[:, gi * GRP:(gi + 1) * GRP]
                    nc.scalar.activation(x, ps[:],
                                         mybir.ActivationFunctionType.Relu)
                    nc.vector.scalar_tensor_tensor(
                        m_row[:, g * GRP:(g + 1) * GRP], x, r2, x,
                        mybir.AluOpType.is_le, mybir.AluOpType.is_ge,
                        accum_out=counts[:, t * NG + g:t * NG + g + 1])
                    nc.vector.scalar_tensor_tensor(
                        x, x, r2, x,
                        mybir.AluOpType.is_le, mybir.AluOpType.mult)
                nc.sync.dma_start(
                    w_d[t * QTILE:(t + 1) * QTILE,
                        h * (N // 2):(h + 1) * (N // 2)], w_half[:])
            nc.sync.dma_start(mask_d[t * QTILE:(t + 1) * QTILE, :], m_row[:])
        nc.sync.dma_start(cnt_d[:], counts[:])
    _split_multiwaits(nc)
    return nc


_cache = {}


def _get_nc(r2: float):
    if r2 not in _cache:
        _cache[r2] = _build(r2)
    return _cache[r2]


_OUT_SPECS = [("mask", (QC, N), np.uint8),
              ("weights", (QC, N), np.float32),
              ("counts", (QTILE, NQT * NG), np.float32)]

_exec_cache = {}


def _get_exec(r2: float):
    """One jitted SPMD callable: concat qd_aug [8*5, QC+N] -> 3 concat outs.
    Output zero-buffers are created in-graph (on device) and donated to the
    bass_exec custom call, so repeat calls upload nothing but the inputs."""
    if r2 in _exec_cache:
        return _exec_cache[r2]
    install_neuronx_cc_hook()
    nc = _get_nc(r2)
    pid_name = nc.partition_id_tensor.name if nc.partition_id_tensor else None
    in_names = ["qd_aug"] + [n for n, _, _ in _OUT_SPECS]
    if pid_name is not None:
        in_names.append(pid_name)
    out_avals = tuple(jax.core.ShapedArray(s, d) for _, s, d in _OUT_SPECS)
    out_names = tuple(n for n, _, _ in _OUT_SPECS)

    def _body(qd, *zeros):
        operands = [qd, *zeros]
        if pid_name is not None:
            operands.append(partition_id_tensor())
        outs = _bass_exec_p.bind(
            *operands,
            out_avals=out_avals,
            in_names=tuple(in_names),
            out_names=out_names,
            lowering_input_output_aliases=(),
            sim_require_finite=True,
            sim_require_nnan=True,
            nc=nc,
        )
        return tuple(outs)

    devices = jax.devices()[:NCORES]
    mesh = Mesh(np.asarray(devices), ("core",))
    nspec = 1 + len(_OUT_SPECS)
    fn = jax.jit(shard_map(
        _body, mesh=mesh, in_specs=(PartitionSpec("core"),) * nspec,
        out_specs=(PartitionSpec("core"),) * len(_OUT_SPECS),
        check_rep=False), donate_argnums=tuple(range(1, nspec)))

    sharding = jax.sharding.NamedSharding(mesh, PartitionSpec("core"))
    make_zeros = jax.jit(
        lambda: tuple(jnp.zeros((NCORES * s[0], *s[1:]), d)
                      for _, s, d in _OUT_SPECS),
        out_shardings=(sharding,) * len(_OUT_SPECS))
    _exec_cache[r2] = (fn, make_zeros)
    return _exec_cache[r2]


def _prep_inputs(data, queries):
    """Concatenated per-core augmented operand matrix [8*5, QC+N]."""
    d2 = (data[:, 0] * data[:, 0] + data[:, 1] * data[:, 1]
          + data[:, 2] * data[:, 2])
    q2 = (queries[:, 0] * queries[:, 0] + queries[:, 1] * queries[:, 1]
          + queries[:, 2] * queries[:, 2])
    d_aug = np.stack([data[:, 0], data[:, 1], data[:, 2],
                      np.ones(N, np.float32), d2]).astype(np.float32)
    blocks = []
    for c in range(NCORES):
        qs = slice(c * QC, (c + 1) * QC)
        q_aug = np.stack([-2.0 * queries[qs, 0], -2.0 * queries[qs, 1],
                          -2.0 * queries[qs, 2], q2[qs],
                          np.ones(QC, np.float32)]).astype(np.float32)
        blocks.append(np.concatenate([q_aug, d_aug], axis=1))
    return np.ascontiguousarray(np.concatenate(blocks, axis=0))


def kernel(data, queries, radius):
    data = np.ascontiguousarray(np.asarray(data, dtype=np.float32))
    queries = np.ascontiguousarray(np.asarray(queries, dtype=np.float32))
    r = np.float32(np.asarray(radius).reshape(()))
    r2 = float(r * r)

    fn, make_zeros = _get_exec(r2)
    qd = _prep_inputs(data, queries)
    mask_c, w_c, cnt_c = fn(qd, *make_zeros())

    mask = np.asarray(mask_c).view(np.bool_)
    weights = np.asarray(w_c)
    cnt = np.asarray(cnt_c).reshape(NCORES, QTILE, NQT, NG)
    # counts[c, p, t, :] are the per-group counts of query c*QC + t*128 + p
    counts = cnt.sum(axis=3).transpose(0, 2, 1).reshape(Q).astype(np.int64)
    row_splits = np.concatenate(
        [np.zeros(1, np.int64), np.cumsum(counts)]).astype(np.int32)
    return mask, row_splits, weights
